# revision 8
# baseline (speedup 1.0000x reference)
"""Trainium2 Bass kernel for nn_DotAttention (B=8 data-parallel over 8 cores).

Per core (one batch element), bf16 with one fp8 DoubleRow stage. v2 layout:
the ACT engine runs the 32 exp instructions back-to-back with no table
switches (RELU projections moved to DVE tensor_scalar, sigmoids deferred
until after the last exp); the PE round-robins [next-pair scores, U(t),
one background quantum] per exp pair so the exp stream never starves.

  x.T/m.T   : bf16 pre-cast (DVE) then bf16 PE transposes (1 cyc/row)
  xp/mp     : W.T @ {x,m}.T into PSUM, bias+relu via DVE tensor_scalar
  S.T       : mp.T(:,jtile) @ xp.T, bf16, K=96
  e8        : exp(S.T*scale + maskbias) -> fp8e4 on ACT (table 0 only)
  U[jx,151] : fp8 DoubleRow vs [m|1]; denominator in col 150
  normalize : reciprocal_approx_fast + per-partition tensor_scalar -> bf16
  gate      : res.T chunks stationary, Wg moving; logits copied to SBUF
  tail      : one table switch, sigmoid per 2-chunk pair, gate*res mults,
              output DMA per pair alternating sync/tensor queues
DMA: sync queue m0..m3+mask+Wg+bg, scalar queue x0..x3 (idle before exps),
gpsimd queue Wi/bi/Wm/bm. PSUM: scores 2x2 banks, U 3, shared small bank.
"""

import contextlib
import math

import numpy as np

import concourse.bass as bass
import concourse.mybir as mybir
import concourse.tile as tile
from concourse import bacc
from concourse.bass_utils import run_bass_kernel_spmd
from concourse.masks import make_identity

F32 = mybir.dt.float32
F16 = mybir.dt.bfloat16
F8 = mybir.dt.float8e4
I32 = mybir.dt.int32
DR = mybir.MatmulPerfMode.DoubleRow

B = 8
JX = 2048
JM = 2048
D = 150
H = 96
G = 300
NJT = 16          # jm tiles of 128
NCH = 16          # jx chunks of 128
HALF = 1024
NSUB = HALF // 512
SCALE = 1.0 / math.sqrt(float(H))
NEG_BIG = 1.0e30


def _body(tc, x_d, m_d, mask_d, wi_d, bi_d, wm_d, bm_d, wg_d, bg_d, o_d):
    nc = tc.nc
    Exp = mybir.ActivationFunctionType.Exp
    Sigmoid = mybir.ActivationFunctionType.Sigmoid
    MUL = mybir.AluOpType.mult
    SUB = mybir.AluOpType.subtract
    ADD = mybir.AluOpType.add
    MAX = mybir.AluOpType.max

    with contextlib.ExitStack() as ctx:
        const = ctx.enter_context(tc.tile_pool(name="const", bufs=1))
        work = ctx.enter_context(tc.tile_pool(name="work", bufs=2))
        epool = ctx.enter_context(tc.tile_pool(name="epool", bufs=3))
        psb = ctx.enter_context(tc.tile_pool(name="psb", bufs=2, space="PSUM"))
        pu = ctx.enter_context(tc.tile_pool(name="pu", bufs=1, space="PSUM"))

        # ---- gpsimd queue head: identities --------------------------------
        ident16 = const.tile([128, 128], F16)
        make_identity(nc, ident16)
        ident32s = const.tile([NJT, NJT], F32)
        make_identity(nc, ident32s)

        # scalar queue head: exp table preload (table_sel=0)
        dummy = const.tile([1, 1], F32)
        nc.scalar.activation(out=dummy, in_=ident16[0:1, 0:1], func=Exp, scale=1.0)

        # ---- input DMAs ---------------------------------------------------
        # sync queue: m groups first (feed the m->scores chain), then mask,
        # then the late-needed Wg/bg.  scalar queue: x groups (ACT is idle
        # until the first exp).  gpsimd queue: Wi/bi/Wm/bm for projections.
        x_nat = const.tile([128, NCH, D], F32)
        m_nat = const.tile([128, NJT, D], F32)
        x_re = x_d.rearrange("(n p) d -> p n d", p=128)
        m_re = m_d.rearrange("(n p) d -> p n d", p=128)
        for g in range(4):
            gs4 = slice(g * 4, (g + 1) * 4)
            nc.sync.dma_start(out=m_nat[:, gs4, :], in_=m_re[:, gs4, :])
            nc.scalar.dma_start(out=x_nat[:, gs4, :], in_=x_re[:, gs4, :])
        mask_sb = const.tile([NJT, 128], I32)
        nc.sync.dma_start(out=mask_sb, in_=mask_d.rearrange("(n p) -> n p", p=128))

        wstage = const.tile([128, 2 * H], F32)
        wstage2 = const.tile([D - 128, 2 * H], F32)
        bi_sb = const.tile([H, 1], F32)
        bm_sb = const.tile([H, 1], F32)
        nc.gpsimd.dma_start(out=wstage[:, 0:H], in_=wi_d[0:128, :])
        nc.gpsimd.dma_start(out=wstage2[:, 0:H], in_=wi_d[128:D, :])
        nc.gpsimd.dma_start(out=bi_sb, in_=bi_d.rearrange("(n one) -> n one", one=1))
        nc.gpsimd.dma_start(out=wstage[:, H : 2 * H], in_=wm_d[0:128, :])
        nc.gpsimd.dma_start(out=wstage2[:, H : 2 * H], in_=wm_d[128:D, :])
        nc.gpsimd.dma_start(out=bm_sb, in_=bm_d.rearrange("(n one) -> n one", one=1))

        # ---- PE warmup: ramp the p-state while the first DMAs land -------
        jp = psb.tile([128, 128], F32, tag="sm", name="junk", bufs=1)
        for _ in range(14):
            nc.tensor.matmul(
                jp, ident16, ident16, start=True, stop=True,
                skip_group_check=True)
        nc.vector.tensor_copy(out=dummy, in_=jp[0:1, 0:1])

        # ---- weight casts to bf16 (vector) -------------------------------
        wi16a = const.tile([128, H], F16)
        nc.vector.tensor_copy(out=wi16a, in_=wstage[:, 0:H])
        wi16b = const.tile([D - 128, H], F16)
        nc.vector.tensor_copy(out=wi16b, in_=wstage2[:, 0:H])
        wm16a = const.tile([128, H], F16)
        nc.vector.tensor_copy(out=wm16a, in_=wstage[:, H : 2 * H])
        wm16b = const.tile([D - 128, H], F16)
        nc.vector.tensor_copy(out=wm16b, in_=wstage2[:, H : 2 * H])

        # ---- bf16 naturals + fp8 m for the U matmuls ----------------------
        x16 = const.tile([128, NCH, D], F16)
        m16 = const.tile([128, NJT, D], F16)
        mt8 = const.tile([128, NJT, 176], F8)
        nc.gpsimd.memset(mt8[:, :, D:176], 0.0)
        nc.gpsimd.memset(mt8[:, :, 150:151], 1.0)

        def cast_group(g, which):
            gs4 = slice(g * 4, (g + 1) * 4)
            if which == "x":
                nc.vector.tensor_copy(out=x16[:, gs4, :], in_=x_nat[:, gs4, :])
            else:
                nc.vector.tensor_copy(out=m16[:, gs4, :], in_=m_nat[:, gs4, :])
                eng = nc.vector if g == 0 else nc.gpsimd
                eng.tensor_copy(out=mt8[:, gs4, 0:D], in_=m_nat[:, gs4, :])

        # ---- mask -> additive exp bias [128, NJT] ------------------------
        maskf = const.tile([NJT, 128], F32)
        nc.vector.tensor_copy(out=maskf, in_=mask_sb)
        nc.vector.tensor_scalar(
            out=maskf, in0=maskf, scalar1=1.0, scalar2=NEG_BIG,
            op0=SUB, op1=MUL)
        mb_ps = psb.tile([128, NJT], F32, tag="sm", name="mbps", bufs=1)
        nc.tensor.transpose(mb_ps, maskf, ident32s)
        maskbias = const.tile([128, NJT], F32)
        nc.vector.tensor_copy(out=maskbias, in_=mb_ps)

        # ---- transposed bf16 layouts --------------------------------------
        xT16a = const.tile([128, JX], F16)
        mT16a = const.tile([128, JM], F16)
        mT16b = const.tile([D - 128, JM], F16)
        # merged tail: x.T tail rows 0..21, U.T tail rows 32..53, ones row 64
        rtail = const.tile([65, JX], F16)
        nc.vector.memset(rtail, 0.0)
        nc.vector.memset(rtail[64:65, :], 1.0)

        def transpose_group(src16, dstA, dstB, g):
            # one 4-chunk group (512 cols) as two bf16 2-chunk pieces
            for p2 in range(2):
                pT = psb.tile([128, 2, 256], F16, tag="sm", name="pT", bufs=1)
                for i in range(2):
                    c = g * 4 + p2 * 2 + i
                    nc.tensor.transpose(
                        pT[:, i, 0:128], src16[:, c, 0:128], ident16)
                    nc.tensor.transpose(
                        pT[0 : D - 128, i, 128:256], src16[:, c, 128:D],
                        ident16)
                gcols = slice(g * 512 + p2 * 256, g * 512 + (p2 + 1) * 256)
                nc.vector.tensor_copy(out=dstA[:, gcols], in_=pT[:, :, 0:128])
                nc.vector.tensor_copy(
                    out=dstB[0 : D - 128, gcols],
                    in_=pT[0 : D - 128, :, 128:256])

        # ---- projections: matmul into PSUM, bias+relu on DVE -------------
        xpT16 = const.tile([H, JX], F16)
        mpT16 = const.tile([H, JM], F16)

        def proj_sub(wa, wb, srcA, srcB, b_sb, dst, sub):
            ss = slice(sub * 512, (sub + 1) * 512)
            pp = psb.tile([H, 512], F32, tag="sm", name="pp", bufs=1)
            nc.tensor.matmul(
                pp, wa, srcA[:, ss],
                start=True, stop=False, skip_group_check=True)
            nc.tensor.matmul(
                pp, wb, srcB[0 : D - 128, ss],
                start=False, stop=True, skip_group_check=True)
            nc.vector.tensor_scalar(
                out=dst[:, ss], in0=pp, scalar1=b_sb, scalar2=0.0,
                op0=ADD, op1=MAX)

        def do_group(g, which):
            cast_group(g, which)
            if which == "x":
                transpose_group(x16, xT16a, rtail, g)
                proj_sub(wi16a, wi16b, xT16a, rtail, bi_sb, xpT16, g)
            else:
                transpose_group(m16, mT16a, mT16b, g)
                proj_sub(wm16a, wm16b, mT16a, mT16b, bm_sb, mpT16, g)

        # ---- Wg/bg: stage f32 (sync DMA), cast on vector mid-window ------
        wg16a = const.tile([128, G], F16, tag="wg16a")
        wg16c = const.tile([128, G], F16, tag="wg16c")
        wgtail = const.tile([65, G], F16, tag="wgtail")
        nc.gpsimd.memset(wgtail, 0.0)
        wg_stages = []
        for sl, (g0, g1), w, r0 in ((0, (0, 128), wg16a, 0),
                                    (1, (128, 150), wgtail, 0),
                                    (2, (150, 278), wg16c, 0),
                                    (3, (278, 300), wgtail, 32)):
            wst = const.tile([g1 - g0, G], F32, tag=f"wgst_{sl}", name=f"wgst{sl}")
            nc.sync.dma_start(out=wst, in_=wg_d[g0:g1, :])
            wg_stages.append((wst, w, r0, g1 - g0))
        bgst = const.tile([1, G], F32, tag="bgst")
        nc.sync.dma_start(out=bgst, in_=bg_d.rearrange("(one n) -> one n", one=1))

        def cast_wg():
            for wst, w, r0, rows in wg_stages:
                nc.vector.tensor_copy(out=w[r0 : r0 + rows, :], in_=wst)
            nc.vector.tensor_copy(out=wgtail[64:65, :], in_=bgst)

        # ---- attention state ---------------------------------------------
        U16n = const.tile([128, NCH, 160], F16)
        nc.vector.memset(U16n[:, :, 150:160], 0.0)
        rcp_all = const.tile([128, NCH], F32)
        uT16a = const.tile([128, JX], F16)
        glog = const.tile([128, NCH, G], F32)
        gate16 = const.tile([128, NCH, G], F16)
        o_re = o_d.rearrange("(n p) k -> p n k", p=128)

        def ut_group(g):
            # transpose U16n chunks 2g, 2g+1 into uT16a / rtail rows 32..53
            pA = psb.tile([128, 2, 256], F16, tag="sm", name="pUA", bufs=1)
            for i in range(2):
                c = g * 2 + i
                nc.tensor.transpose(
                    pA[:, i, 0:128], U16n[:, c, 0:128], ident16)
                nc.tensor.transpose(
                    pA[0 : D - 128, i, 128:256], U16n[:, c, 128:D], ident16)
            gcols = slice(g * 256, (g + 1) * 256)
            nc.vector.tensor_copy(out=uT16a[:, gcols], in_=pA[:, :, 0:128])
            nc.vector.tensor_copy(
                out=rtail[32 : 32 + D - 128, gcols],
                in_=pA[0 : D - 128, :, 128:256])

        def gate_chunk(c):
            cs = slice(c * 128, (c + 1) * 128)
            gp = psb.tile([128, G], F32, tag="sm", name="gp", bufs=1)
            for gi, (lhs, w) in enumerate((
                (xT16a[:, cs], wg16a), (uT16a[:, cs], wg16c),
                (rtail[:, cs], wgtail))):
                nc.tensor.matmul(
                    gp, lhs, w,
                    start=(gi == 0), stop=(gi == 2), skip_group_check=True)
            nc.vector.tensor_copy(out=glog[:, c, :], in_=gp)

        def sig_pair(cp):
            c2 = slice(cp * 2, cp * 2 + 2)
            nc.scalar.activation(
                out=gate16[:, c2, :], in_=glog[:, c2, :], func=Sigmoid,
                scale=1.0)

        def out_pair(cp, dma_eng):
            c2 = slice(cp * 2, cp * 2 + 2)
            onat = work.tile([128, 2, G], F32, tag="onat", bufs=4)
            # keep the mult off the engine that issues this pair's DMA
            eng = nc.gpsimd if dma_eng is nc.sync else nc.vector
            eng.tensor_tensor(
                out=onat[:, :, 0:D], in0=gate16[:, c2, 0:D],
                in1=x_nat[:, c2, :], op=MUL)
            eng.tensor_tensor(
                out=onat[:, :, D:G], in0=gate16[:, c2, D:G],
                in1=U16n[:, c2, 0:D], op=MUL)
            dma_eng.dma_start(out=o_re[:, c2, :], in_=onat)

        def norm_chunk(c, Up, h):
            nc.vector.tensor_scalar(
                out=U16n[:, c, 0:D], in0=Up[:, c - h * 8, 0:D],
                scalar1=rcp_all[:, c : c + 1],
                scalar2=None, op0=MUL)

        def emit_scores(h, j):
            sp = psb.tile([128, HALF], F32, tag="big", name="sp")
            for sx in range(NSUB):
                ss = slice(h * HALF + sx * 512, h * HALF + (sx + 1) * 512)
                nc.tensor.matmul(
                    sp[:, sx * 512 : (sx + 1) * 512],
                    mpT16[:, j * 128 : (j + 1) * 128], xpT16[:, ss],
                    start=True, stop=True, skip_group_check=True)
            return sp

        # ---- preamble: x g0,g1 + m g0 -> first scores --------------------
        state = {"mg": 0, "xg": 0}

        def need_m(jtiles):
            while state["mg"] * 4 < jtiles:
                do_group(state["mg"], "m")
                state["mg"] += 1

        def need_x(chunks):
            while state["xg"] * 4 < chunks:
                do_group(state["xg"], "x")
                state["xg"] += 1

        need_x(8)
        need_m(4)

        # background quanta; bg_h0[t] runs at the END of h0 slot t so the
        # group is emitted before the slot-(t+1) scores that first read it
        bg_h0 = [
            lambda: need_m(8),
            lambda: need_m(12),
            lambda: need_x(12),
            lambda: need_m(16),
            lambda: need_x(16),
            lambda: cast_wg(),
        ]

        # ---- attention main loop -----------------------------------------
        sps = [emit_scores(0, 0), emit_scores(0, 1)]
        Ups = [None, None]
        for h in range(2):
            Up = pu.tile([128, 8, 171], F32, tag="U", name="Up")
            Ups[h] = Up
            e_cur = epool.tile([128, 2, HALF], F8, tag="e8", name="e8")
            for t in range(NJT // 2):
                for s in range(2):
                    j = 2 * t + s
                    nc.scalar.activation(
                        out=e_cur[:, s, :], in_=sps[s], func=Exp,
                        bias=maskbias[:, j : j + 1], scale=SCALE)
                # next-pair scores first: keep the exp stream fed
                if t < NJT // 2 - 1:
                    need_m(2 * t + 4)
                    sps = [emit_scores(h, 2 * t + 2),
                           emit_scores(h, 2 * t + 3)]
                elif h == 0:
                    need_x(16)
                    sps = [emit_scores(1, 0), emit_scores(1, 1)]
                for c in range(8):
                    nc.tensor.matmul(
                        Up[:, c, 0:151],
                        e_cur[:, :, c * 128 : (c + 1) * 128],
                        mt8[:, 2 * t : 2 * t + 2, 0:151],
                        start=(t == 0), stop=(t == NJT // 2 - 1),
                        perf_mode=DR, skip_group_check=True)
                # one background quantum per slot
                if h == 0:
                    if t < len(bg_h0):
                        bg_h0[t]()
                else:
                    if t == 0:
                        # h0 normalize (DVE) while PE streams h1 scores
                        hc = slice(0, 8)
                        den = work.tile([128, 8], F32, tag="den")
                        nc.vector.tensor_copy(out=den, in_=Ups[0][:, :, 150])
                        nc.vector.reciprocal_approx_fast(
                            out=rcp_all[:, hc], in_=den)
                        for c in range(8):
                            norm_chunk(c, Ups[0], 0)
                        ut_group(0)
                    elif t == 1:
                        gate_chunk(0)
                        gate_chunk(1)
                        ut_group(1)
                    elif t == 2:
                        gate_chunk(2)
                        gate_chunk(3)
                        ut_group(2)
                    elif t == 3:
                        gate_chunk(4)
                        gate_chunk(5)
                        ut_group(3)
                    elif t == 4:
                        gate_chunk(6)
                        gate_chunk(7)
                if t < NJT // 2 - 1:
                    e_cur = epool.tile([128, 2, HALF], F8, tag="e8", name="e8")

        # ---- tail ---------------------------------------------------------
        # h1 normalize
        den = work.tile([128, 8], F32, tag="den")
        nc.vector.tensor_copy(out=den, in_=Ups[1][:, :, 150])
        nc.vector.reciprocal_approx_fast(out=rcp_all[:, 8:16], in_=den)
        for c in range(8, 16):
            norm_chunk(c, Ups[1], 1)

        # sigmoids for h0 chunks can start immediately (one table switch)
        sig_pair(0)
        sig_pair(1)
        out_pair(0, nc.sync)
        ut_group(4)
        gate_chunk(8)
        gate_chunk(9)
        sig_pair(2)
        out_pair(1, nc.gpsimd)
        ut_group(5)
        gate_chunk(10)
        gate_chunk(11)
        sig_pair(3)
        out_pair(2, nc.sync)
        ut_group(6)
        gate_chunk(12)
        gate_chunk(13)
        sig_pair(4)
        out_pair(3, nc.gpsimd)
        ut_group(7)
        gate_chunk(14)
        gate_chunk(15)
        sig_pair(5)
        out_pair(4, nc.sync)
        sig_pair(6)
        out_pair(5, nc.gpsimd)
        sig_pair(7)
        out_pair(6, nc.sync)
        out_pair(7, nc.gpsimd)


_NC_CACHE = None


def _build_nc():
    global _NC_CACHE
    if _NC_CACHE is not None:
        return _NC_CACHE
    nc = bacc.Bacc(None, target_bir_lowering=False, debug=False)
    x_d = nc.dram_tensor("x", [JX, D], F32, kind="ExternalInput")
    m_d = nc.dram_tensor("m", [JM, D], F32, kind="ExternalInput")
    mask_d = nc.dram_tensor("mask", [JM], I32, kind="ExternalInput")
    wi_d = nc.dram_tensor("Wi", [D, H], F32, kind="ExternalInput")
    bi_d = nc.dram_tensor("bi", [H], F32, kind="ExternalInput")
    wm_d = nc.dram_tensor("Wm", [D, H], F32, kind="ExternalInput")
    bm_d = nc.dram_tensor("bm", [H], F32, kind="ExternalInput")
    wg_d = nc.dram_tensor("Wg", [G, G], F32, kind="ExternalInput")
    bg_d = nc.dram_tensor("bg", [G], F32, kind="ExternalInput")
    o_d = nc.dram_tensor("out", [JX, G], F32, kind="ExternalOutput")
    with tile.TileContext(nc) as tc:
        _body(tc, x_d, m_d, mask_d, wi_d, bi_d, wm_d, bm_d, wg_d, bg_d, o_d)
    nc.finalize()
    _NC_CACHE = nc
    return nc


def _in_maps(inputs, memory, mask, Wi, bi, Wm, bm, Wg, bg):
    maps = []
    for b in range(B):
        maps.append(
            {
                "x": np.ascontiguousarray(inputs[b], dtype=np.float32),
                "m": np.ascontiguousarray(memory[b], dtype=np.float32),
                "mask": np.ascontiguousarray(mask[b], dtype=np.int32),
                "Wi": np.ascontiguousarray(Wi, dtype=np.float32),
                "bi": np.ascontiguousarray(bi, dtype=np.float32),
                "Wm": np.ascontiguousarray(Wm, dtype=np.float32),
                "bm": np.ascontiguousarray(bm, dtype=np.float32),
                "Wg": np.ascontiguousarray(Wg, dtype=np.float32),
                "bg": np.ascontiguousarray(bg, dtype=np.float32),
            }
        )
    return maps


def run_spmd(inputs, memory, mask, Wi, bi, Wm, bm, Wg, bg, **spmd_kwargs):
    """Run the kernel across 8 cores; returns the BassKernelResults."""
    nc = _build_nc()
    maps = _in_maps(
        np.asarray(inputs), np.asarray(memory), np.asarray(mask),
        np.asarray(Wi), np.asarray(bi), np.asarray(Wm), np.asarray(bm),
        np.asarray(Wg), np.asarray(bg),
    )
    return run_bass_kernel_spmd(nc, maps, list(range(B)), **spmd_kwargs)


def kernel(inputs, memory, mask, Wi, bi, Wm, bm, Wg, bg):
    res = run_spmd(inputs, memory, mask, Wi, bi, Wm, bm, Wg, bg)
    out = np.stack([res.results[b]["out"] for b in range(B)], axis=0)
    return out.astype(np.float32)
